# revision 6
# baseline (speedup 1.0000x reference)
"""Bag-of-words histogram kernel for Trainium2 (Bass/Tile), 8-core data-parallel.

Problem: docs [256, 2048] int32 token ids in [0, 32000) ->
         hist [256, 32000] fp32, hist[b, v] = count(docs[b, :] == v) / 2048.

Algorithm (per core, 32 rows):
  Factor each token t = 256*hi + lo (hi < 125, lo < 256). Then
    hist[b, hi, lo] = sum_s onehot_hi[s, hi] * onehot_lo[s, lo]
  computed as bf16 one-hot outer products on the PE, accumulated in PSUM
  over 16 k-tiles of 128 tokens per row.

  One-hot construction: measured HW cost of a DVE op is ~150-230ns fixed
  + ~0.25-0.5ns/elem, so per-k-tile tensor_scalar builds (the baseline's
  structure; 1024 ops) are fixed-cost dominated. Instead, ONE
  tensor_tensor is_equal per (row, side) builds all 16 k-tiles' one-hots
  at once in an interleaved [128, bins, 16] layout:
      out[p, c, k] = (iota_rep[p, c*16+k] == tok_side[p, k])
  with in0 a materialized repeated-iota tile (value c at flat position
  c*16+k, unit stride) and in1 the per-k-tile targets broadcast along the
  bin dim (stride-0 middle dim, unit last-dim stride keeps the DVE 2x
  mode). 64 wide DVE ops replace 1024 narrow ones.

  The matmul for k-tile k reads the strided [128, bins] slice [:, :, k].
  PSUM accumulates counts; the ACT engine applies the 1/2048 scale on the
  PSUM->SBUF copy; DMA writes each row's [125, 256] fp32 tile to HBM.

Sharding: batch axis split 8 ways (32 rows per core), no communication.
"""

import sys

import numpy as np

for _p in ("/opt/trn_rl_repo",):
    if _p not in sys.path:
        sys.path.append(_p)

BATCH = 256
SEQ = 2048
VOCAB = 32000
N_CORES = 8
ROWS = BATCH // N_CORES  # 32 rows per core
P = 128
KT = SEQ // P            # 16 k-tiles per row
GR = 8                   # rows per input-DMA group
NLO = 256                # lo = t & 255
NHI = 128                # hi = t >> 8 < 125, padded to 128


def _build_nc():
    from contextlib import ExitStack

    from concourse import bacc, bass, mybir
    from concourse.tile import TileContext

    nc = bacc.Bacc()
    docs = nc.dram_tensor("docs", [ROWS, SEQ], mybir.dt.int32, kind="ExternalInput")
    hist = nc.dram_tensor("hist", [ROWS, VOCAB], mybir.dt.float32, kind="ExternalOutput")

    f32 = mybir.dt.float32
    bf16 = mybir.dt.bfloat16
    i32 = mybir.dt.int32
    Alu = mybir.AluOpType

    with TileContext(nc) as tc, ExitStack() as ctx:
        const_tp = ctx.enter_context(tc.tile_pool(name="const", bufs=1))
        tok_tp = ctx.enter_context(tc.tile_pool(name="tok", bufs=4))
        sc_tp = ctx.enter_context(tc.tile_pool(name="sc", bufs=4))
        oh_tp = ctx.enter_context(tc.tile_pool(name="oh", bufs=3))
        res_tp = ctx.enter_context(tc.tile_pool(name="res", bufs=4))
        psum_tp = ctx.enter_context(tc.tile_pool(name="psum", bufs=8, space="PSUM"))

        # Repeated-iota constants: value c at flat position c*KT + k.
        iota_lo = const_tp.tile([P, NLO, KT], bf16)
        nc.gpsimd.iota(iota_lo[:], [[1, NLO], [0, KT]], channel_multiplier=0,
                       allow_small_or_imprecise_dtypes=True)
        iota_hi = const_tp.tile([P, NHI, KT], bf16)
        nc.gpsimd.iota(iota_hi[:], [[1, NHI], [0, KT]], channel_multiplier=0,
                       allow_small_or_imprecise_dtypes=True)

        for g in range(ROWS // GR):
            # Load GR rows; partition p holds tokens [16p, 16p+16) of each row
            # (any within-row permutation is histogram-invariant).
            tok = tok_tp.tile([P, GR, KT], i32)
            src = bass.AP(docs, g * GR * SEQ, [[16, P], [SEQ, GR], [1, KT]])
            nc.sync.dma_start(out=tok[:], in_=src)

            # Per-token compare targets as exact bf16 values.
            lo_i = sc_tp.tile([P, GR, KT], i32, tag="loi")
            nc.vector.tensor_scalar(out=lo_i[:], in0=tok[:], scalar1=255,
                                    scalar2=None, op0=Alu.bitwise_and)
            lo_f = sc_tp.tile([P, GR, KT], bf16, tag="lof")
            nc.vector.tensor_scalar(out=lo_f[:], in0=lo_i[:], scalar1=1.0,
                                    scalar2=None, op0=Alu.mult)
            hi_i = sc_tp.tile([P, GR, KT], i32, tag="hii")
            nc.vector.tensor_scalar(out=hi_i[:], in0=tok[:], scalar1=8,
                                    scalar2=None, op0=Alu.logical_shift_right)
            hi_f = sc_tp.tile([P, GR, KT], bf16, tag="hif")
            nc.vector.tensor_scalar(out=hi_f[:], in0=hi_i[:], scalar1=1.0,
                                    scalar2=None, op0=Alu.mult)

            for rl in range(GR):
                r = g * GR + rl
                # Build all 16 k-tiles' one-hots for this row in 2 DVE ops.
                oh_lo = oh_tp.tile([P, NLO, KT], bf16, tag="ohlo")
                nc.vector.tensor_tensor(
                    out=oh_lo[:], in0=iota_lo[:],
                    in1=lo_f[:, rl:rl + 1, :].to_broadcast([P, NLO, KT]),
                    op=Alu.is_equal)
                oh_hi = oh_tp.tile([P, NHI, KT], bf16, tag="ohhi")
                nc.vector.tensor_tensor(
                    out=oh_hi[:], in0=iota_hi[:],
                    in1=hi_f[:, rl:rl + 1, :].to_broadcast([P, NHI, KT]),
                    op=Alu.is_equal)

                ps = psum_tp.tile([P, NLO], f32)
                for k in range(KT):
                    nc.tensor.matmul(out=ps[:], lhsT=oh_hi[:, :, k],
                                     rhs=oh_lo[:, :, k],
                                     start=(k == 0), stop=(k == KT - 1))

                res = res_tp.tile([P, NLO], f32)
                nc.scalar.mul(out=res[:], in_=ps[:], mul=1.0 / SEQ)
                nc.sync.dma_start(
                    out=hist[r].rearrange("(h l) -> h l", l=NLO),
                    in_=res[:VOCAB // NLO, :])
    nc.compile()
    return nc


_NC_CACHE = None


def _get_nc():
    global _NC_CACHE
    if _NC_CACHE is None:
        _NC_CACHE = _build_nc()
    return _NC_CACHE


def run_sharded(docs: np.ndarray, trace: bool = False):
    """Run the 8-core SPMD kernel. Returns (full_output, BassKernelResults)."""
    from concourse.bass_utils import run_bass_kernel_spmd

    docs = np.ascontiguousarray(np.asarray(docs, dtype=np.int32))
    assert docs.shape == (BATCH, SEQ), docs.shape
    shards = docs.reshape(N_CORES, ROWS, SEQ)
    in_maps = [{"docs": shards[i]} for i in range(N_CORES)]
    res = run_bass_kernel_spmd(_get_nc(), in_maps, core_ids=list(range(N_CORES)),
                               trace=trace)
    out = np.concatenate([res.results[i]["hist"] for i in range(N_CORES)], axis=0)
    return out, res


def kernel(docs: np.ndarray) -> np.ndarray:
    out, _ = run_sharded(docs, trace=False)
    return out


# revision 17
# speedup vs baseline: 1.9194x; 1.9194x over previous
"""Bag-of-words histogram kernel for Trainium2 (Bass/Tile), 8-core data-parallel.

Problem: docs [256, 2048] int32 token ids in [0, 32000) ->
         hist [256, 32000] fp32, hist[b, v] = count(docs[b, :] == v) / 2048.

Algorithm (per core, 32 rows):
  Factor each token t = 256*hi + lo (hi < 125, lo < 256). Then
    hist[b, hi, lo] = sum_s onehot_hi[s, hi] * onehot_lo[s, lo]
  computed as bf16 one-hot outer products on the PE, accumulated in PSUM
  over 16 k-tiles of 128 tokens per row.

  Engine assignment (from microbenchmarks on this part):
  - PE: needs unit- or stride-2 rhs (moving side); lhsT tolerates stride.
    bf16 matmul ~140-165ns effective when fed back-to-back.
  - DVE: ~150-230ns fixed/op + 0.25ns/elem (tensor_scalar 4x) or
    0.56ns/elem (tensor_tensor 2x). Wide batched TT ops win.
  So:
  - hi one-hots (lhsT): ONE TT is_equal per row builds all 16 k-tiles in
    interleaved [128, 128, 16] layout; matmuls read stride-16 weights.
  - lo one-hots (rhs): ONE TT is_equal per ROW-PAIR and k-tile builds
    [128, 256, 2] (two rows side by side); matmuls read stride-2 slices.
    A tunable fraction of k-tiles is built per-row on ACT (|d| ->
    relu(1-d)) instead, to balance the two engines.
  - PSUM->SBUF 1/2048-scaled copies run on the otherwise idle GPSIMD.
  DMA writes each row's [125, 256] fp32 tile straight to HBM.

Sharding: batch axis split 8 ways (32 rows per core), no communication.
"""

import sys

import numpy as np

for _p in ("/opt/trn_rl_repo",):
    if _p not in sys.path:
        sys.path.append(_p)

BATCH = 256
SEQ = 2048
VOCAB = 32000
N_CORES = 8
ROWS = BATCH // N_CORES  # 32 rows per core
P = 128
KT = SEQ // P            # 16 k-tiles per row
GR = 8                   # rows per input-DMA group
NLO = 256                # lo = t & 255
NHI = 128                # hi = t >> 8 < 125, padded to 128

# k-tiles whose lo one-hot is built per-row on the ACT engine.
ACT_KS = frozenset({3, 7, 11, 15})
# k-tiles whose lo one-hot pair is built on GPSIMD instead of DVE.
GP_KS = frozenset()
# Engine for the PSUM->SBUF scaled copy: "G" gpsimd, "A" scalar.
COPY_ENGINE = "G"


def _build_nc():
    from contextlib import ExitStack

    from concourse import bacc, bass, mybir
    from concourse.tile import TileContext

    nc = bacc.Bacc()
    docs = nc.dram_tensor("docs", [ROWS, SEQ], mybir.dt.int32, kind="ExternalInput")
    hist = nc.dram_tensor("hist", [ROWS, VOCAB], mybir.dt.float32, kind="ExternalOutput")

    f32 = mybir.dt.float32
    bf16 = mybir.dt.bfloat16
    i32 = mybir.dt.int32
    Alu = mybir.AluOpType
    Act = mybir.ActivationFunctionType

    with TileContext(nc) as tc, ExitStack() as ctx:
        const_tp = ctx.enter_context(tc.tile_pool(name="const", bufs=1))
        tok_tp = ctx.enter_context(tc.tile_pool(name="tok", bufs=4))
        sc_tp = ctx.enter_context(tc.tile_pool(name="sc", bufs=4))
        ohh_tp = ctx.enter_context(tc.tile_pool(name="ohh", bufs=4))
        ohl_tp = ctx.enter_context(tc.tile_pool(name="ohl", bufs=20))
        res_tp = ctx.enter_context(tc.tile_pool(name="res", bufs=4))
        psum_tp = ctx.enter_context(tc.tile_pool(name="psum", bufs=8, space="PSUM"))

        # iota constants
        iota_hi = const_tp.tile([P, NHI, KT], bf16)   # value c at c*KT+k
        nc.gpsimd.iota(iota_hi[:], [[1, NHI], [0, KT]], channel_multiplier=0,
                       allow_small_or_imprecise_dtypes=True)
        iota_lo2 = const_tp.tile([P, NLO, 2], bf16)   # value c at 2c, 2c+1
        nc.gpsimd.iota(iota_lo2[:], [[1, NLO], [0, 2]], channel_multiplier=0,
                       allow_small_or_imprecise_dtypes=True)
        iota_lo = const_tp.tile([P, NLO], bf16)       # 0..255 (ACT path)
        nc.gpsimd.iota(iota_lo[:], [[1, NLO]], channel_multiplier=0,
                       allow_small_or_imprecise_dtypes=True)
        one_bias = const_tp.tile([P, 1], f32)         # ACT relu bias constant
        nc.gpsimd.memset(one_bias[:], 1.0)

        for g in range(ROWS // GR):
            # Load GR rows; partition p holds tokens [16p, 16p+16) of each row
            # (any within-row permutation is histogram-invariant).
            tok = tok_tp.tile([P, GR, KT], i32)
            src = bass.AP(docs, g * GR * SEQ, [[16, P], [SEQ, GR], [1, KT]])
            nc.sync.dma_start(out=tok[:], in_=src)


            # Extract+cast compare targets (int op then cast-mult; a fused
            # int-op0/float-op1 tensor_scalar fails backend codegen).
            hi_i = sc_tp.tile([P, GR, KT], i32, tag="hii")
            nc.vector.tensor_scalar(out=hi_i[:], in0=tok[:], scalar1=8,
                                    scalar2=None, op0=Alu.logical_shift_right)
            hi_f = sc_tp.tile([P, GR, KT], bf16, tag="hif")
            nc.vector.tensor_scalar(out=hi_f[:], in0=hi_i[:], scalar1=1.0,
                                    scalar2=None, op0=Alu.mult)
            lo_i = sc_tp.tile([P, GR, KT], i32, tag="loi")
            nc.vector.tensor_scalar(out=lo_i[:], in0=tok[:], scalar1=255,
                                    scalar2=None, op0=Alu.bitwise_and)
            lo_it = lo_i[:].transpose([0, 2, 1])
            # lo_ft [P, KT, GR] bf16 (row-adjacent, for the pair TT).
            lo_ft = sc_tp.tile([P, KT, GR], bf16, tag="loft")
            nc.vector.tensor_scalar(out=lo_ft[:], in0=lo_it, scalar1=1.0,
                                    scalar2=None, op0=Alu.mult)
            if ACT_KS:
                # -lo as fp32 [P, KT, GR] for the ACT bias operand.
                nlo_ft = sc_tp.tile([P, KT, GR], f32, tag="nloft")
                nc.vector.tensor_scalar(out=nlo_ft[:], in0=lo_it, scalar1=-1.0,
                                        scalar2=None, op0=Alu.mult)

            for rl in range(GR):
                r = g * GR + rl
                # All 16 hi one-hots for this row in one DVE op (k-inner).
                oh_hi = ohh_tp.tile([P, NHI, KT], bf16)
                nc.vector.tensor_tensor(
                    out=oh_hi[:], in0=iota_hi[:],
                    in1=hi_f[:, rl:rl + 1, :].to_broadcast([P, NHI, KT]),
                    op=Alu.is_equal)

                if rl % 2 == 0:
                    # Build this row-pair's lo one-hots (DVE k-tiles only).
                    lo_pair = {}
                    for k in range(KT):
                        if k in ACT_KS:
                            continue
                        t2 = ohl_tp.tile([P, NLO, 2], bf16, tag="ohlo")
                        eng = nc.gpsimd if k in GP_KS else nc.vector
                        eng.tensor_tensor(
                            out=t2[:], in0=iota_lo2[:],
                            in1=lo_ft[:, k:k + 1, rl:rl + 2].to_broadcast(
                                [P, NLO, 2]),
                            op=Alu.is_equal)
                        lo_pair[k] = t2

                ps = psum_tp.tile([P, NLO], f32)
                for k in range(KT):
                    if k in ACT_KS:
                        d = ohl_tp.tile([P, NLO], bf16, tag="dabs")
                        nc.scalar.activation(
                            out=d[:], in_=iota_lo[:], func=Act.Abs,
                            bias=nlo_ft[:, k, rl:rl + 1], scale=1.0)
                        oh_lo = ohl_tp.tile([P, NLO], bf16, tag="ohloa")
                        nc.scalar.activation(
                            out=oh_lo[:], in_=d[:], func=Act.Relu,
                            bias=one_bias[:], scale=-1.0)
                        rhs = oh_lo[:]
                    else:
                        rhs = lo_pair[k][:, :, rl % 2]
                    nc.tensor.matmul(out=ps[:], lhsT=oh_hi[:, :, k],
                                     rhs=rhs,
                                     start=(k == 0), stop=(k == KT - 1))

                res = res_tp.tile([P, NLO], f32)
                if COPY_ENGINE == "G":
                    nc.gpsimd.tensor_scalar(out=res[:], in0=ps[:],
                                            scalar1=1.0 / SEQ, scalar2=None,
                                            op0=Alu.mult)
                else:
                    nc.scalar.mul(out=res[:], in_=ps[:], mul=1.0 / SEQ)
                nc.sync.dma_start(
                    out=hist[r].rearrange("(h l) -> h l", l=NLO),
                    in_=res[:VOCAB // NLO, :])
    nc.compile()
    return nc


_NC_CACHE = None


def _get_nc():
    global _NC_CACHE
    if _NC_CACHE is None:
        _NC_CACHE = _build_nc()
    return _NC_CACHE


def run_sharded(docs: np.ndarray, trace: bool = False):
    """Run the 8-core SPMD kernel. Returns (full_output, BassKernelResults)."""
    from concourse.bass_utils import run_bass_kernel_spmd

    docs = np.ascontiguousarray(np.asarray(docs, dtype=np.int32))
    assert docs.shape == (BATCH, SEQ), docs.shape
    shards = docs.reshape(N_CORES, ROWS, SEQ)
    in_maps = [{"docs": shards[i]} for i in range(N_CORES)]
    res = run_bass_kernel_spmd(_get_nc(), in_maps, core_ids=list(range(N_CORES)),
                               trace=trace)
    out = np.concatenate([res.results[i]["hist"] for i in range(N_CORES)], axis=0)
    return out, res


def kernel(docs: np.ndarray) -> np.ndarray:
    out, _ = run_sharded(docs, trace=False)
    return out


# revision 18
# speedup vs baseline: 2.0500x; 1.0681x over previous
"""Bag-of-words histogram kernel for Trainium2 (Bass/Tile), 8-core data-parallel.

Problem: docs [256, 2048] int32 token ids in [0, 32000) ->
         hist [256, 32000] fp32, hist[b, v] = count(docs[b, :] == v) / 2048.

Algorithm (per core, 32 rows):
  Factor each token t = 256*hi + lo (hi < 125, lo < 256). Then
    hist[b, hi, lo] = sum_s onehot_hi[s, hi] * onehot_lo[s, lo]
  computed as bf16 one-hot outer products on the PE, accumulated in PSUM
  over 16 k-tiles of 128 tokens per row.

  Engine assignment (from microbenchmarks on this part):
  - PE: needs unit- or stride-2 rhs (moving side); lhsT tolerates stride.
    bf16 matmul ~140-165ns effective when fed back-to-back.
  - DVE: ~150-230ns fixed/op + 0.25ns/elem (tensor_scalar 4x) or
    0.56ns/elem (tensor_tensor 2x). Wide batched TT ops win.
  So:
  - hi one-hots (lhsT): ONE TT is_equal per row builds all 16 k-tiles in
    interleaved [128, 128, 16] layout; matmuls read stride-16 weights.
  - lo one-hots (rhs): ONE TT is_equal per ROW-PAIR and k-tile builds
    [128, 256, 2] (two rows side by side); matmuls read stride-2 slices.
    A tunable fraction of k-tiles is built per-row on ACT (|d| ->
    relu(1-d)) instead, to balance the two engines.
  - PSUM->SBUF 1/2048-scaled copies run on the otherwise idle GPSIMD.
  DMA writes each row's [125, 256] fp32 tile straight to HBM.

Sharding: batch axis split 8 ways (32 rows per core), no communication.
"""

import sys

import numpy as np

for _p in ("/opt/trn_rl_repo",):
    if _p not in sys.path:
        sys.path.append(_p)

BATCH = 256
SEQ = 2048
VOCAB = 32000
N_CORES = 8
ROWS = BATCH // N_CORES  # 32 rows per core
P = 128
KT = SEQ // P            # 16 k-tiles per row
GR = 8                   # rows per input-DMA group
NLO = 256                # lo = t & 255
NHI = 128                # hi = t >> 8 < 125, padded to 128

# k-tiles whose lo one-hot is built per-row on the ACT engine.
ACT_KS = frozenset({5, 10, 15})
# k-tiles whose lo one-hot pair is built on GPSIMD instead of DVE.
GP_KS = frozenset()
# Engine for the PSUM->SBUF scaled copy: "G" gpsimd, "A" scalar.
COPY_ENGINE = "G"


def _build_nc():
    from contextlib import ExitStack

    from concourse import bacc, bass, mybir
    from concourse.tile import TileContext

    nc = bacc.Bacc()
    docs = nc.dram_tensor("docs", [ROWS, SEQ], mybir.dt.int32, kind="ExternalInput")
    hist = nc.dram_tensor("hist", [ROWS, VOCAB], mybir.dt.float32, kind="ExternalOutput")

    f32 = mybir.dt.float32
    bf16 = mybir.dt.bfloat16
    i32 = mybir.dt.int32
    Alu = mybir.AluOpType
    Act = mybir.ActivationFunctionType

    with TileContext(nc) as tc, ExitStack() as ctx:
        const_tp = ctx.enter_context(tc.tile_pool(name="const", bufs=1))
        tok_tp = ctx.enter_context(tc.tile_pool(name="tok", bufs=4))
        sc_tp = ctx.enter_context(tc.tile_pool(name="sc", bufs=4))
        ohh_tp = ctx.enter_context(tc.tile_pool(name="ohh", bufs=6))
        ohl_tp = ctx.enter_context(tc.tile_pool(name="ohl", bufs=40))
        res_tp = ctx.enter_context(tc.tile_pool(name="res", bufs=8))
        psum_tp = ctx.enter_context(tc.tile_pool(name="psum", bufs=8, space="PSUM"))

        # iota constants
        iota_hi = const_tp.tile([P, NHI, KT], bf16)   # value c at c*KT+k
        nc.gpsimd.iota(iota_hi[:], [[1, NHI], [0, KT]], channel_multiplier=0,
                       allow_small_or_imprecise_dtypes=True)
        iota_lo2 = const_tp.tile([P, NLO, 2], bf16)   # value c at 2c, 2c+1
        nc.gpsimd.iota(iota_lo2[:], [[1, NLO], [0, 2]], channel_multiplier=0,
                       allow_small_or_imprecise_dtypes=True)
        iota_lo = const_tp.tile([P, NLO], bf16)       # 0..255 (ACT path)
        nc.gpsimd.iota(iota_lo[:], [[1, NLO]], channel_multiplier=0,
                       allow_small_or_imprecise_dtypes=True)
        one_bias = const_tp.tile([P, 1], f32)         # ACT relu bias constant
        nc.gpsimd.memset(one_bias[:], 1.0)

        for g in range(ROWS // GR):
            # Load GR rows; partition p holds tokens [16p, 16p+16) of each row
            # (any within-row permutation is histogram-invariant).
            tok = tok_tp.tile([P, GR, KT], i32)
            src = bass.AP(docs, g * GR * SEQ, [[16, P], [SEQ, GR], [1, KT]])
            nc.sync.dma_start(out=tok[:], in_=src)


            # Extract+cast compare targets (int op then cast-mult; a fused
            # int-op0/float-op1 tensor_scalar fails backend codegen).
            hi_i = sc_tp.tile([P, GR, KT], i32, tag="hii")
            nc.vector.tensor_scalar(out=hi_i[:], in0=tok[:], scalar1=8,
                                    scalar2=None, op0=Alu.logical_shift_right)
            hi_f = sc_tp.tile([P, GR, KT], bf16, tag="hif")
            nc.vector.tensor_scalar(out=hi_f[:], in0=hi_i[:], scalar1=1.0,
                                    scalar2=None, op0=Alu.mult)
            lo_i = sc_tp.tile([P, GR, KT], i32, tag="loi")
            nc.vector.tensor_scalar(out=lo_i[:], in0=tok[:], scalar1=255,
                                    scalar2=None, op0=Alu.bitwise_and)
            lo_it = lo_i[:].transpose([0, 2, 1])
            # lo_ft [P, KT, GR] bf16 (row-adjacent, for the pair TT).
            lo_ft = sc_tp.tile([P, KT, GR], bf16, tag="loft")
            nc.vector.tensor_scalar(out=lo_ft[:], in0=lo_it, scalar1=1.0,
                                    scalar2=None, op0=Alu.mult)
            if ACT_KS:
                # -lo as fp32 [P, KT, GR] for the ACT bias operand.
                nlo_ft = sc_tp.tile([P, KT, GR], f32, tag="nloft")
                nc.vector.tensor_scalar(out=nlo_ft[:], in0=lo_it, scalar1=-1.0,
                                        scalar2=None, op0=Alu.mult)

            for rl in range(GR):
                r = g * GR + rl
                # All 16 hi one-hots for this row in one DVE op (k-inner).
                oh_hi = ohh_tp.tile([P, NHI, KT], bf16)
                nc.vector.tensor_tensor(
                    out=oh_hi[:], in0=iota_hi[:],
                    in1=hi_f[:, rl:rl + 1, :].to_broadcast([P, NHI, KT]),
                    op=Alu.is_equal)

                if rl % 2 == 0:
                    # Build this row-pair's lo one-hots (DVE k-tiles only).
                    lo_pair = {}
                    for k in range(KT):
                        if k in ACT_KS:
                            continue
                        t2 = ohl_tp.tile([P, NLO, 2], bf16, tag="ohlo")
                        eng = nc.gpsimd if k in GP_KS else nc.vector
                        eng.tensor_tensor(
                            out=t2[:], in0=iota_lo2[:],
                            in1=lo_ft[:, k:k + 1, rl:rl + 2].to_broadcast(
                                [P, NLO, 2]),
                            op=Alu.is_equal)
                        lo_pair[k] = t2

                ps = psum_tp.tile([P, NLO], f32)
                for k in range(KT):
                    if k in ACT_KS:
                        d = ohl_tp.tile([P, NLO], bf16, tag="dabs")
                        nc.scalar.activation(
                            out=d[:], in_=iota_lo[:], func=Act.Abs,
                            bias=nlo_ft[:, k, rl:rl + 1], scale=1.0)
                        oh_lo = ohl_tp.tile([P, NLO], bf16, tag="ohloa")
                        nc.scalar.activation(
                            out=oh_lo[:], in_=d[:], func=Act.Relu,
                            bias=one_bias[:], scale=-1.0)
                        rhs = oh_lo[:]
                    else:
                        rhs = lo_pair[k][:, :, rl % 2]
                    nc.tensor.matmul(out=ps[:], lhsT=oh_hi[:, :, k],
                                     rhs=rhs,
                                     start=(k == 0), stop=(k == KT - 1))

                res = res_tp.tile([P, NLO], f32)
                if COPY_ENGINE == "G":
                    nc.gpsimd.tensor_scalar(out=res[:], in0=ps[:],
                                            scalar1=1.0 / SEQ, scalar2=None,
                                            op0=Alu.mult)
                else:
                    nc.scalar.mul(out=res[:], in_=ps[:], mul=1.0 / SEQ)
                nc.sync.dma_start(
                    out=hist[r].rearrange("(h l) -> h l", l=NLO),
                    in_=res[:VOCAB // NLO, :])
    nc.compile()
    return nc


_NC_CACHE = None


def _get_nc():
    global _NC_CACHE
    if _NC_CACHE is None:
        _NC_CACHE = _build_nc()
    return _NC_CACHE


def run_sharded(docs: np.ndarray, trace: bool = False):
    """Run the 8-core SPMD kernel. Returns (full_output, BassKernelResults)."""
    from concourse.bass_utils import run_bass_kernel_spmd

    docs = np.ascontiguousarray(np.asarray(docs, dtype=np.int32))
    assert docs.shape == (BATCH, SEQ), docs.shape
    shards = docs.reshape(N_CORES, ROWS, SEQ)
    in_maps = [{"docs": shards[i]} for i in range(N_CORES)]
    res = run_bass_kernel_spmd(_get_nc(), in_maps, core_ids=list(range(N_CORES)),
                               trace=trace)
    out = np.concatenate([res.results[i]["hist"] for i in range(N_CORES)], axis=0)
    return out, res


def kernel(docs: np.ndarray) -> np.ndarray:
    out, _ = run_sharded(docs, trace=False)
    return out
